# revision 25
# baseline (speedup 1.0000x reference)
"""Bass/Trainium2 kernel for LightweightHypersphericalAttention (v3).

Sharding: 8 cores = (batch b in 0..3) x (query half in 0..1).
Each core gets x_sh [1024, 512] (its query rows), ctx [2048, 512] (full
context for its batch), the weights, and radius; computes its [1024, 512]
slice of the final output. No collectives; host concatenates slices.

Structure:
  - Chunked hwdge f32 loads + compute-engine casts to bf16 naturals,
    then PE transposes (bf16, 4 blocks batched per psum + one strided
    copy) for W/Wp/x/ctx.
  - Row norms: bf16 squares (DVE) + one-hot selector matmuls into one
    shared psum; sqrt/recip/radius applied after transposing to the
    full-partition layout (DVE reciprocal is ~8x faster with 128 lanes).
  - fq broadcast across partitions via DRAM bounce (overlaps k proj).
  - Flash loop (nt outer, h inner): S^T in psum, exp on ACT with
    per-partition fk scale and a constant bias shift, PT in fp8e4 so the
    AV and denominator-selector matmuls stream at 4 cols/cycle.
  - Softmax denominator: per (h,j) selector matmul accumulates column
    sums of PT into a [4,512] psum; normalization deferred to the output
    projection where 1/den is a per-partition scalar in natural [n, co]
    orientation (scalar_tensor_tensor accumulate over heads).
  - Output projection of query half 0 interleaved into half 1's flash.
"""

import numpy as np

P = 128
B, N, M, C, H = 4, 2048, 2048, 512, 4
D_V = 128
D_QK = 256
SCALE = float(D_V) ** -0.5
EPS = 1e-12
N_CORE = 1024          # query rows per core
NN = N_CORE // P       # 8 query chunks
MM = M // P            # 16 key chunks
CCH = C // P           # 4 channel chunks
NT = N_CORE // 512     # 2 query 512-tiles
MT = M // 512          # 4 key 512-tiles
FP8 = False            # fp8 PT measured at ~1.4e-2 rel err: too close to 2e-2
SHIFT = 0.0            # exp logit shift (cancels in softmax ratio)

_NC_CACHE = {}


def _build(mm_bf16: bool):
    import concourse.bass as bass
    import concourse.mybir as mybir
    import concourse.tile as tile
    from concourse import bacc
    from concourse.masks import make_identity

    assert mm_bf16, "v3 kernel is bf16-only"
    f32 = mybir.dt.float32
    bf16 = mybir.dt.bfloat16
    f8 = mybir.dt.float8e4 if FP8 else bf16
    add_op = mybir.AluOpType.add
    mult_op = mybir.AluOpType.mult

    nc = bacc.Bacc(None, target_bir_lowering=False, debug=False)
    x_t = nc.dram_tensor("x_sh", [N_CORE, C], f32, kind="ExternalInput")
    c_t = nc.dram_tensor("ctx", [M, C], f32, kind="ExternalInput")
    wq_t = nc.dram_tensor("w_qkv", [2 * C, C], f32, kind="ExternalInput")
    wp_t = nc.dram_tensor("w_proj", [C, C], f32, kind="ExternalInput")
    rad_t = nc.dram_tensor("radius", [H], f32, kind="ExternalInput")
    out_t = nc.dram_tensor("out_sh", [N_CORE, C], f32, kind="ExternalOutput")

    from contextlib import ExitStack
    with tile.TileContext(nc) as tc, ExitStack() as es:
        const = es.enter_context(tc.tile_pool(name="const", bufs=1))
        ld = es.enter_context(tc.tile_pool(name="ld", bufs=8))
        natp = es.enter_context(tc.tile_pool(name="natp", bufs=1))
        wpool = es.enter_context(tc.tile_pool(name="wpool", bufs=1))
        big = es.enter_context(tc.tile_pool(name="big", bufs=1))
        sq = es.enter_context(tc.tile_pool(name="sq", bufs=2))
        fp = es.enter_context(tc.tile_pool(name="fp", bufs=1))
        ptp = es.enter_context(tc.tile_pool(name="ptp", bufs=4))
        outp = es.enter_context(tc.tile_pool(name="outp", bufs=2))
        ps_m = es.enter_context(tc.tile_pool(name="ps_m", bufs=2, space="PSUM"))
        ps_ss = es.enter_context(tc.tile_pool(name="ps_ss", bufs=1, space="PSUM"))
        ps_s = es.enter_context(tc.tile_pool(name="ps_s", bufs=3, space="PSUM"))
        ps_av = es.enter_context(tc.tile_pool(name="ps_av", bufs=2, space="PSUM"))

        # ---- constants ----
        identity = const.tile([P, P], f32)
        make_identity(nc, identity)
        identity_bf = const.tile([P, P], bf16)
        nc.vector.tensor_copy(out=identity_bf, in_=identity)
        rad_ap = rad_t[:]
        rad_b = const.tile([P, H], f32)
        nc.sync.dma_start(
            out=rad_b, in_=bass.AP(tensor=rad_ap.tensor, offset=rad_ap.offset,
                                   ap=[[0, P], rad_ap.ap[0]]))
        rad_s = const.tile([P, H], f32)
        nc.scalar.mul(out=rad_s, in_=rad_b, mul=SCALE)
        # Dbc[:, r, :]: [8, 128] stationary that broadcasts fq row r to
        # all 128 output partitions (contraction over the 8 fq rows).

        # ---- chunked load + cast + PE-transpose helpers ----
        cast_engines = [
            lambda out, in_: nc.vector.tensor_copy(out=out, in_=in_),
            lambda out, in_: nc.scalar.copy(out=out, in_=in_),
        ]
        cast_i = 0

        def load_cast(dram_ap, nat_tile, chunks):
            nonlocal cast_i
            for ch in range(0, chunks, 2):
                raw = ld.tile([P, 2, C], f32, tag="ld")
                nc.sync.dma_start(out=raw, in_=dram_ap[:, ch:ch + 2, :])
                cast_engines[cast_i % len(cast_engines)](
                    nat_tile[:, ch:ch + 2, :], raw)
                cast_i += 1

        def transpose_chunks(nat_tile, T_tile, chunks):
            # T_tile[:, cc, ch*128:+128] = nat_tile[:, ch, cc*128:+128].T
            for ch in range(chunks):
                pst = ps_m.tile([P, CCH, P], bf16, tag="m",
                                name=f"pst_{T_tile.tensor.name}_{ch}")
                for cc in range(CCH):
                    nc.tensor.transpose(pst[:, cc, :],
                                        nat_tile[:, ch, cc * P:(cc + 1) * P],
                                        identity_bf)
                if ch % 2 == 0:
                    nc.vector.tensor_copy(
                        out=T_tile[:, :, ch * P:(ch + 1) * P], in_=pst)
                else:
                    nc.scalar.copy(
                        out=T_tile[:, :, ch * P:(ch + 1) * P], in_=pst)

        w_bf = natp.tile([P, 2 * C // P, C], bf16)
        load_cast(wq_t[:].rearrange("(oo p) c -> p oo c", p=P), w_bf,
                  2 * C // P)
        WT = wpool.tile([P, CCH, 2 * C], bf16)
        transpose_chunks(w_bf, WT, 2 * C // P)

        x_bf = natp.tile([P, NN, C], bf16)
        load_cast(x_t[:].rearrange("(nn p) c -> p nn c", p=P), x_bf, NN)
        xT = big.tile([P, CCH, N_CORE], bf16, tag="xT")
        transpose_chunks(x_bf, xT, NN)

        Dq = const.tile([P, NT * H, NT * H], bf16)
        nc.vector.memset(Dq, 0.0)
        for r in range(NT * H):
            nc.vector.memset(Dq[:, r, r:r + 1], 1.0)
        Dk = const.tile([P, MT * H, MT * H], bf16)
        nc.vector.memset(Dk, 0.0)
        for r in range(MT * H):
            nc.vector.memset(Dk[:, r, r:r + 1], 1.0)
        Dden = const.tile([P, H, H], f8)
        nc.vector.memset(Dden, 0.0)
        for r in range(H):
            nc.vector.memset(Dden[:, r, r:r + 1], 1.0)
        Dbc = const.tile([NT * H, NT * H, P], bf16)
        for r in range(NT * H):
            nc.vector.tensor_copy(
                out=Dbc[:, r, :],
                in_=identity[0:NT * H, r:r + 1].to_broadcast((NT * H, P)))
        Dbc4 = const.tile([H, H, P], bf16)
        for r in range(H):
            nc.vector.tensor_copy(
                out=Dbc4[:, r, :],
                in_=identity[0:H, r:r + 1].to_broadcast((H, P)))
        bias_t = const.tile([P, 1], f32)
        nc.vector.memset(bias_t, -SHIFT)

        # ---- q projection: qT[d, do, n] (all heads) ----
        qT = big.tile([P, 2 * H, N_CORE], bf16, tag="qT")
        for do in range(2 * H):
            for nt in range(NT):
                psq = ps_m.tile([P, 512], f32, tag="m")
                for cc in range(CCH):
                    nc.tensor.matmul(
                        psq, WT[:, cc, do * P:(do + 1) * P],
                        xT[:, cc, nt * 512:(nt + 1) * 512],
                        start=(cc == 0), stop=(cc == CCH - 1))
                nc.vector.tensor_copy(
                    out=qT[:, do, nt * 512:(nt + 1) * 512], in_=psq)

        # ctx load/transpose overlaps q-ss on PE; v = ctx_bf slices (bf16)
        ctx_bf = natp.tile([P, MM, C], bf16)
        load_cast(c_t[:].rearrange("(mm p) c -> p mm c", p=P), ctx_bf, MM)
        cT = big.tile([P, CCH, M], bf16, tag="cT")
        transpose_chunks(ctx_bf, cT, MM)
        v_sb = ctx_bf

        # ---- q sum-of-squares -> fq (rows r = 2h + nt) ----
        ps_ssq = ps_ss.tile([NT * H, 512], f32, tag="ss")
        n_ssq = H * NT * 2
        i_ssq = 0
        for h in range(H):
            sqt = sq.tile([P, 2, N_CORE], bf16, tag="sq")
            nc.vector.tensor_tensor(sqt, qT[:, 2 * h:2 * h + 2, :],
                                    qT[:, 2 * h:2 * h + 2, :], mult_op)
            for nt in range(NT):
                for dc in range(2):
                    nc.tensor.matmul(
                        ps_ssq, Dq[:, 2 * h + nt, :],
                        sqt[:, dc, nt * 512:(nt + 1) * 512],
                        start=(i_ssq == 0), stop=(i_ssq == n_ssq - 1),
                        skip_group_check=True)
                    i_ssq += 1
        ssq_sb = fp.tile([NT * H, 512], f32, tag="ssq_sb")
        nc.scalar.copy(out=ssq_sb, in_=ps_ssq)
        ps_tq = ps_m.tile([P, 4, NT * H], f32, tag="m")
        for b in range(4):
            nc.tensor.transpose(ps_tq[:, b, :],
                                ssq_sb[:, b * P:(b + 1) * P],
                                identity[:NT * H, :NT * H])
        # fq = r_h / max(sqrt(ss), eps) in [n-part, b, r] layout
        fqn = fp.tile([P, 4, NT * H], f32, tag="fqn")
        nc.scalar.activation(fqn, ps_tq, mybir.ActivationFunctionType.Sqrt)
        nc.vector.tensor_scalar_max(fqn, fqn, EPS)
        nc.vector.reciprocal(fqn, fqn)
        fqn_v = fqn.rearrange("p b (h nt) -> p b h nt", nt=NT)
        nc.vector.tensor_tensor(
            fqn_v, fqn_v,
            rad_b[:, None, :, None].to_broadcast((P, 4, H, NT)), mult_op)
        # ---- k projection: kT[d, do, m] ----
        kT = big.tile([P, 2 * H, M], bf16, tag="kT")
        fq_rows = None

        def kproj_do(do):
            for mt in range(MT):
                psk = ps_m.tile([P, 512], f32, tag="m",
                                name=f"psk{do}{mt}")
                for cc in range(CCH):
                    nc.tensor.matmul(
                        psk, WT[:, cc, do * P:(do + 1) * P],
                        cT[:, cc, mt * 512:(mt + 1) * 512],
                        start=(cc == 0), stop=(cc == CCH - 1))
                nc.scalar.copy(
                    out=kT[:, do, mt * 512:(mt + 1) * 512], in_=psk)

        for do in range(2):
            kproj_do(do)
        # transpose fq back to row layout (fqn chain has drained by now)
        ps_fr = ps_m.tile([NT * H, 4, P], f32, tag="m")
        for b in range(4):
            nc.tensor.transpose(ps_fr[:, b, :], fqn[:, b, :], identity)
        fq_rows = fp.tile([NT * H, 4, P], bf16, tag="fq_rows")
        nc.scalar.copy(out=fq_rows, in_=ps_fr)
        for do in range(2, 2 * H):
            kproj_do(do)

        # ---- k sum-of-squares -> fk (rows r = 4h + mt) ----
        ps_ssk = ps_ss.tile([MT * H, 512], f32, tag="ss")
        n_ssk = H * MT * 2
        i_ssk = 0
        for h in range(H):
            sqt = sq.tile([P, 2, M], bf16, tag="sqk")
            nc.vector.tensor_tensor(sqt, kT[:, 2 * h:2 * h + 2, :],
                                    kT[:, 2 * h:2 * h + 2, :], mult_op)
            for mt in range(MT):
                for dc in range(2):
                    nc.tensor.matmul(
                        ps_ssk, Dk[:, MT * h + mt, :],
                        sqt[:, dc, mt * 512:(mt + 1) * 512],
                        start=(i_ssk == 0), stop=(i_ssk == n_ssk - 1),
                        skip_group_check=True)
                    i_ssk += 1
        ssk_sb = fp.tile([MT * H, 512], f32, tag="ssk_sb")
        nc.scalar.copy(out=ssk_sb, in_=ps_ssk)
        ps_tk = ps_m.tile([P, 4, MT * H], f32, tag="m")
        for b in range(4):
            nc.tensor.transpose(ps_tk[:, b, :],
                                ssk_sb[:, b * P:(b + 1) * P],
                                identity[:MT * H, :MT * H])
        # fk = r_h * SCALE / max(sqrt(ss), eps) in [key-part, b, r] layout
        fkn = fp.tile([P, 4, MT * H], f32, tag="fkn")
        nc.scalar.activation(fkn, ps_tk, mybir.ActivationFunctionType.Sqrt)
        nc.vector.tensor_scalar_max(fkn, fkn, EPS)
        nc.vector.reciprocal(fkn, fkn)
        fkn_v = fkn.rearrange("p b (h mt) -> p b h mt", mt=MT)
        nc.vector.tensor_tensor(
            fkn_v, fkn_v,
            rad_s[:, None, :, None].to_broadcast((P, 4, H, MT)), mult_op)

        # wp load/transpose (needed only by the output projection)
        wp_bf = natp.tile([P, CCH, C], bf16)
        load_cast(wp_t[:].rearrange("(oo p) c -> p oo c", p=P), wp_bf, CCH)
        WpT = wpool.tile([P, CCH, C], bf16)
        transpose_chunks(wp_bf, WpT, CCH)

        # scale qT in place: q_hat = q_raw * fq. fq is broadcast across
        # partitions with a 1-contraction ones matmul (no DRAM bounce).
        for h in range(H):
            for nt in range(NT):
                ps_b = ps_m.tile([P, 512], f32, tag="m",
                                 name=f"psb{h}{nt}")
                nc.tensor.matmul(
                    ps_b, Dbc[:, 2 * h + nt, :],
                    fq_rows.rearrange("r b p -> r (b p)"),
                    start=True, stop=True)
                nc.vector.tensor_tensor(
                    qT[:, 2 * h:2 * h + 2, nt * 512:(nt + 1) * 512],
                    qT[:, 2 * h:2 * h + 2, nt * 512:(nt + 1) * 512],
                    ps_b[:, None, :].to_broadcast((P, 2, 512)), mult_op)

        outcatT = big.tile([P, H, N_CORE], bf16, tag="ocT")
        rdens = [None, None]

        def outproj_block(nt, b):
            # outcatT is already normalized: plain psum accumulation over h
            nn = nt * 4 + b
            ps_o = ps_m.tile([P, C], f32, tag="m", name=f"pso{nt}{b}")
            for h in range(H):
                nc.tensor.matmul(ps_o, outcatT[:, h, nn * P:(nn + 1) * P],
                                 WpT[:, h, :], start=(h == 0),
                                 stop=(h == H - 1))
            acc_o = outp.tile([P, C], f32, tag="acco")
            nc.scalar.copy(out=acc_o, in_=ps_o)
            nc.scalar.dma_start(out=out_t[nn * P:(nn + 1) * P, :],
                                in_=acc_o)

        def scale_outcat(nt):
            # outcatT[:, h, nt half] *= 1/den broadcast across partitions
            rden = rdens[nt]
            ps_rr = ps_m.tile([H, 4, P], f32, tag="m", name=f"psrr{nt}")
            for b in range(4):
                nc.tensor.transpose(ps_rr[:, b, :], rden[:, b, :], identity)
            rr = fp.tile([H, 4, P], bf16, tag="rr", name=f"rr{nt}")
            nc.scalar.copy(out=rr, in_=ps_rr)
            for h in range(H):
                ps_b = ps_m.tile([P, 512], f32, tag="m",
                                 name=f"psbo{nt}{h}")
                nc.tensor.matmul(ps_b, Dbc4[:, h, :],
                                 rr.rearrange("r b p -> r (b p)"),
                                 start=True, stop=True)
                nc.vector.tensor_tensor(
                    outcatT[:, h, nt * 512:(nt + 1) * 512],
                    outcatT[:, h, nt * 512:(nt + 1) * 512],
                    ps_b, mult_op)

        def flash_half(nt, after_head=None):
            den_ps = ps_ss.tile([H, 512], f32, tag="ss", name=f"den{nt}")
            i_den = 0
            for h in range(H):
                avo = ps_av.tile([P, 512], f32, tag="av", name=f"avo{nt}{h}")
                for j in range(MM):
                    psS = ps_s.tile([P, 512], f32, tag="s")
                    nc.tensor.matmul(psS, kT[:, 2 * h, j * P:(j + 1) * P],
                                     qT[:, 2 * h, nt * 512:(nt + 1) * 512],
                                     start=True, stop=False)
                    nc.tensor.matmul(psS,
                                     kT[:, 2 * h + 1, j * P:(j + 1) * P],
                                     qT[:, 2 * h + 1,
                                        nt * 512:(nt + 1) * 512],
                                     start=False, stop=True)
                    PT = ptp.tile([P, 512], f8, tag="pt")
                    nc.scalar.activation(
                        PT, psS, mybir.ActivationFunctionType.Exp,
                        bias=bias_t[:, 0:1],
                        scale=fkn[:, j % 4, MT * h + j // 4:MT * h + j // 4
                                  + 1])
                    nc.tensor.matmul(avo, v_sb[:, j, h * P:(h + 1) * P], PT,
                                     start=(j == 0), stop=(j == MM - 1))
                    nc.tensor.matmul(den_ps, Dden[:, h, :], PT,
                                     start=(i_den == 0),
                                     stop=(i_den == H * MM - 1),
                                     skip_group_check=True)
                    i_den += 1
                nc.vector.tensor_copy(
                    out=outcatT[:, h, nt * 512:(nt + 1) * 512], in_=avo)
                if after_head is not None:
                    after_head(h)
            # 1/den, transposed to natural [n] per-partition orientation
            den_sb = fp.tile([H, 512], f32, tag="den_sb", name=f"densb{nt}")
            nc.scalar.copy(out=den_sb, in_=den_ps)
            ps_td = ps_m.tile([P, 4, H], f32, tag="m", name=f"pstd{nt}")
            for b in range(4):
                nc.tensor.transpose(ps_td[:, b, :],
                                    den_sb[:, b * P:(b + 1) * P],
                                    identity[:H, :H])
            rden = fp.tile([P, 4, H], f32, tag="rden", name=f"rden{nt}")
            nc.vector.reciprocal(rden, ps_td)
            rdens[nt] = rden

        def after_head_nt1(h):
            if h == 0:
                scale_outcat(0)
            outproj_block(0, h)

        flash_half(0)
        # interleave half 0's normalize+projection into half 1's flash
        flash_half(1, after_head=after_head_nt1)
        scale_outcat(1)
        for b in range(4):
            outproj_block(1, b)

    nc.compile()
    return nc


def _get_nc(mm_bf16: bool):
    if mm_bf16 not in _NC_CACHE:
        _NC_CACHE[mm_bf16] = _build(mm_bf16)
    return _NC_CACHE[mm_bf16]


def kernel(x, context, W_qkv, W_proj, radius, _trace=False, _bf16=True):
    from concourse.bass_utils import run_bass_kernel_spmd

    x = np.ascontiguousarray(np.asarray(x, dtype=np.float32))
    context = np.ascontiguousarray(np.asarray(context, dtype=np.float32))
    W_qkv = np.ascontiguousarray(np.asarray(W_qkv, dtype=np.float32))
    W_proj = np.ascontiguousarray(np.asarray(W_proj, dtype=np.float32))
    radius = np.ascontiguousarray(np.asarray(radius, dtype=np.float32))

    nc = _get_nc(True)
    in_maps = []
    for i in range(8):
        b, half = i // 2, i % 2
        in_maps.append({
            "x_sh": x[b, half * N_CORE:(half + 1) * N_CORE, :],
            "ctx": context[b],
            "w_qkv": W_qkv,
            "w_proj": W_proj,
            "radius": radius,
        })
    res = run_bass_kernel_spmd(nc, in_maps, list(range(8)), trace=_trace)
    out = np.empty((B, N, C), dtype=np.float32)
    for i in range(8):
        b, half = i // 2, i % 2
        out[b, half * N_CORE:(half + 1) * N_CORE, :] = res.results[i]["out_sh"]
    if _trace:
        return out, res
    return out


# revision 26
# speedup vs baseline: 1.1911x; 1.1911x over previous
"""Bass/Trainium2 kernel for LightweightHypersphericalAttention (v3).

Sharding: 8 cores = (batch b in 0..3) x (query half in 0..1).
Each core gets x_sh [1024, 512] (its query rows), ctx [2048, 512] (full
context for its batch), the weights, and radius; computes its [1024, 512]
slice of the final output. No collectives; host concatenates slices.

Structure:
  - Chunked hwdge f32 loads + compute-engine casts to bf16 naturals,
    then PE transposes (bf16, 4 blocks batched per psum + one strided
    copy) for W/Wp/x/ctx.
  - Row norms: bf16 squares (DVE) + one-hot selector matmuls into one
    shared psum; sqrt/recip/radius applied after transposing to the
    full-partition layout (DVE reciprocal is ~8x faster with 128 lanes).
  - fq broadcast across partitions via DRAM bounce (overlaps k proj).
  - Flash loop (nt outer, h inner): S^T in psum, exp on ACT with
    per-partition fk scale and a constant bias shift, PT in fp8e4 so the
    AV and denominator-selector matmuls stream at 4 cols/cycle.
  - Softmax denominator: per (h,j) selector matmul accumulates column
    sums of PT into a [4,512] psum; normalization deferred to the output
    projection where 1/den is a per-partition scalar in natural [n, co]
    orientation (scalar_tensor_tensor accumulate over heads).
  - Output projection of query half 0 interleaved into half 1's flash.
"""

import numpy as np

P = 128
B, N, M, C, H = 4, 2048, 2048, 512, 4
D_V = 128
D_QK = 256
SCALE = float(D_V) ** -0.5
EPS = 1e-12
N_CORE = 1024          # query rows per core
NN = N_CORE // P       # 8 query chunks
MM = M // P            # 16 key chunks
CCH = C // P           # 4 channel chunks
NT = N_CORE // 512     # 2 query 512-tiles
MT = M // 512          # 4 key 512-tiles
SHIFT = 1.0            # exp logit shift: keeps fp16 PT under 65504 even at
                       # the theoretical |logit| <= r^2*SCALE = 11.31 bound

_NC_CACHE = {}


def _build(mm_bf16: bool):
    import concourse.bass as bass
    import concourse.mybir as mybir
    import concourse.tile as tile
    from concourse import bacc
    from concourse.masks import make_identity

    assert mm_bf16, "v3 kernel is bf16-only"
    f32 = mybir.dt.float32
    bf16 = mybir.dt.bfloat16
    f16 = mybir.dt.float16
    add_op = mybir.AluOpType.add
    mult_op = mybir.AluOpType.mult

    nc = bacc.Bacc(None, target_bir_lowering=False, debug=False)
    x_t = nc.dram_tensor("x_sh", [N_CORE, C], f32, kind="ExternalInput")
    c_t = nc.dram_tensor("ctx", [M, C], f32, kind="ExternalInput")
    wq_t = nc.dram_tensor("w_qkv", [2 * C, C], f32, kind="ExternalInput")
    wp_t = nc.dram_tensor("w_proj", [C, C], f32, kind="ExternalInput")
    rad_t = nc.dram_tensor("radius", [H], f32, kind="ExternalInput")
    out_t = nc.dram_tensor("out_sh", [N_CORE, C], f32, kind="ExternalOutput")

    from contextlib import ExitStack
    with tile.TileContext(nc) as tc, ExitStack() as es:
        const = es.enter_context(tc.tile_pool(name="const", bufs=1))
        ld = es.enter_context(tc.tile_pool(name="ld", bufs=8))
        natp = es.enter_context(tc.tile_pool(name="natp", bufs=1))
        wpool = es.enter_context(tc.tile_pool(name="wpool", bufs=1))
        big = es.enter_context(tc.tile_pool(name="big", bufs=1))
        sq = es.enter_context(tc.tile_pool(name="sq", bufs=2))
        fp = es.enter_context(tc.tile_pool(name="fp", bufs=1))
        ptp = es.enter_context(tc.tile_pool(name="ptp", bufs=4))
        outp = es.enter_context(tc.tile_pool(name="outp", bufs=2))
        accp = es.enter_context(tc.tile_pool(name="accp", bufs=2))
        ps_m = es.enter_context(tc.tile_pool(name="ps_m", bufs=2, space="PSUM"))
        ps_ss = es.enter_context(tc.tile_pool(name="ps_ss", bufs=1, space="PSUM"))
        ps_s = es.enter_context(tc.tile_pool(name="ps_s", bufs=3, space="PSUM"))
        ps_av = es.enter_context(tc.tile_pool(name="ps_av", bufs=2, space="PSUM"))

        # ---- constants ----
        identity = const.tile([P, P], f32)
        make_identity(nc, identity)
        identity_bf = const.tile([P, P], bf16)
        nc.vector.tensor_copy(out=identity_bf, in_=identity)
        rad_ap = rad_t[:]
        rad_b = const.tile([P, H], f32)
        nc.sync.dma_start(
            out=rad_b, in_=bass.AP(tensor=rad_ap.tensor, offset=rad_ap.offset,
                                   ap=[[0, P], rad_ap.ap[0]]))
        rad_s = const.tile([P, H], f32)
        nc.scalar.mul(out=rad_s, in_=rad_b, mul=SCALE)
        # Dbc[:, r, :]: [8, 128] stationary that broadcasts fq row r to
        # all 128 output partitions (contraction over the 8 fq rows).

        # ---- chunked load + cast + PE-transpose helpers ----
        cast_engines = [
            lambda out, in_: nc.vector.tensor_copy(out=out, in_=in_),
            lambda out, in_: nc.scalar.copy(out=out, in_=in_),
        ]
        cast_i = 0

        def load_cast(dram_ap, nat_tile, chunks):
            nonlocal cast_i
            for ch in range(0, chunks, 2):
                raw = ld.tile([P, 2, C], f32, tag="ld")
                nc.sync.dma_start(out=raw, in_=dram_ap[:, ch:ch + 2, :])
                cast_engines[cast_i % len(cast_engines)](
                    nat_tile[:, ch:ch + 2, :], raw)
                cast_i += 1

        def transpose_chunks(nat_tile, T_tile, chunks):
            # T_tile[:, cc, ch*128:+128] = nat_tile[:, ch, cc*128:+128].T
            for ch in range(chunks):
                pst = ps_m.tile([P, CCH, P], bf16, tag="m",
                                name=f"pst_{T_tile.tensor.name}_{ch}")
                for cc in range(CCH):
                    nc.tensor.transpose(pst[:, cc, :],
                                        nat_tile[:, ch, cc * P:(cc + 1) * P],
                                        identity_bf)
                if ch % 2 == 0:
                    nc.vector.tensor_copy(
                        out=T_tile[:, :, ch * P:(ch + 1) * P], in_=pst)
                else:
                    nc.scalar.copy(
                        out=T_tile[:, :, ch * P:(ch + 1) * P], in_=pst)

        w_bf = natp.tile([P, 2 * C // P, C], bf16)
        load_cast(wq_t[:].rearrange("(oo p) c -> p oo c", p=P), w_bf,
                  2 * C // P)
        WT = wpool.tile([P, CCH, 2 * C], bf16)
        transpose_chunks(w_bf, WT, 2 * C // P)

        x_bf = natp.tile([P, NN, C], bf16)
        load_cast(x_t[:].rearrange("(nn p) c -> p nn c", p=P), x_bf, NN)
        xT = big.tile([P, CCH, N_CORE], bf16, tag="xT")
        transpose_chunks(x_bf, xT, NN)

        Dq = const.tile([P, NT * H, NT * H], bf16)
        nc.vector.memset(Dq, 0.0)
        for r in range(NT * H):
            nc.vector.memset(Dq[:, r, r:r + 1], 1.0)
        Dk = const.tile([P, MT * H, MT * H], bf16)
        nc.vector.memset(Dk, 0.0)
        for r in range(MT * H):
            nc.vector.memset(Dk[:, r, r:r + 1], 1.0)
        Dden = const.tile([P, H, H], f16)
        nc.vector.memset(Dden, 0.0)
        for r in range(H):
            nc.vector.memset(Dden[:, r, r:r + 1], 1.0)
        Dbc = const.tile([NT * H, NT * H, P], bf16)
        for r in range(NT * H):
            nc.vector.tensor_copy(
                out=Dbc[:, r, :],
                in_=identity[0:NT * H, r:r + 1].to_broadcast((NT * H, P)))
        Dbc4 = const.tile([H, H, P], bf16)
        for r in range(H):
            nc.vector.tensor_copy(
                out=Dbc4[:, r, :],
                in_=identity[0:H, r:r + 1].to_broadcast((H, P)))
        bias_t = const.tile([P, 1], f32)
        nc.vector.memset(bias_t, -SHIFT)

        # ---- q projection: qT[d, do, n] (all heads) ----
        qT = big.tile([P, 2 * H, N_CORE], bf16, tag="qT")
        for do in range(2 * H):
            for nt in range(NT):
                psq = ps_m.tile([P, 512], f32, tag="m")
                for cc in range(CCH):
                    nc.tensor.matmul(
                        psq, WT[:, cc, do * P:(do + 1) * P],
                        xT[:, cc, nt * 512:(nt + 1) * 512],
                        start=(cc == 0), stop=(cc == CCH - 1))
                nc.vector.tensor_copy(
                    out=qT[:, do, nt * 512:(nt + 1) * 512], in_=psq)

        # ctx load/transpose overlaps q-ss on PE; v = ctx_bf slices (bf16)
        ctx_bf = natp.tile([P, MM, C], bf16)
        load_cast(c_t[:].rearrange("(mm p) c -> p mm c", p=P), ctx_bf, MM)
        cT = big.tile([P, CCH, M], bf16, tag="cT")
        transpose_chunks(ctx_bf, cT, MM)
        v_sb = ctx_bf

        # ---- q sum-of-squares -> fq (rows r = 2h + nt) ----
        ps_ssq = ps_ss.tile([NT * H, 512], f32, tag="ss")
        n_ssq = H * NT * 2
        i_ssq = 0
        for h in range(H):
            sqt = sq.tile([P, 2, N_CORE], bf16, tag="sq")
            nc.vector.tensor_tensor(sqt, qT[:, 2 * h:2 * h + 2, :],
                                    qT[:, 2 * h:2 * h + 2, :], mult_op)
            for nt in range(NT):
                for dc in range(2):
                    nc.tensor.matmul(
                        ps_ssq, Dq[:, 2 * h + nt, :],
                        sqt[:, dc, nt * 512:(nt + 1) * 512],
                        start=(i_ssq == 0), stop=(i_ssq == n_ssq - 1),
                        skip_group_check=True)
                    i_ssq += 1
        ssq_sb = fp.tile([NT * H, 512], f32, tag="ssq_sb")
        nc.scalar.copy(out=ssq_sb, in_=ps_ssq)
        ps_tq = ps_m.tile([P, 4, NT * H], f32, tag="m")
        for b in range(4):
            nc.tensor.transpose(ps_tq[:, b, :],
                                ssq_sb[:, b * P:(b + 1) * P],
                                identity[:NT * H, :NT * H])
        # fq = r_h / max(sqrt(ss), eps) in [n-part, b, r] layout
        fqn = fp.tile([P, 4, NT * H], f32, tag="fqn")
        nc.scalar.activation(fqn, ps_tq, mybir.ActivationFunctionType.Sqrt)
        nc.vector.tensor_scalar_max(fqn, fqn, EPS)
        nc.vector.reciprocal(fqn, fqn)
        fqn_v = fqn.rearrange("p b (h nt) -> p b h nt", nt=NT)
        nc.vector.tensor_tensor(
            fqn_v, fqn_v,
            rad_b[:, None, :, None].to_broadcast((P, 4, H, NT)), mult_op)
        # ---- k projection: kT[d, do, m] ----
        kT = big.tile([P, 2 * H, M], bf16, tag="kT")
        fq_rows = None

        def kproj_do(do):
            for mt in range(MT):
                psk = ps_m.tile([P, 512], f32, tag="m",
                                name=f"psk{do}{mt}")
                for cc in range(CCH):
                    nc.tensor.matmul(
                        psk, WT[:, cc, do * P:(do + 1) * P],
                        cT[:, cc, mt * 512:(mt + 1) * 512],
                        start=(cc == 0), stop=(cc == CCH - 1))
                nc.scalar.copy(
                    out=kT[:, do, mt * 512:(mt + 1) * 512], in_=psk)

        for do in range(2):
            kproj_do(do)
        # transpose fq back to row layout (fqn chain has drained by now)
        ps_fr = ps_m.tile([NT * H, 4, P], f32, tag="m")
        for b in range(4):
            nc.tensor.transpose(ps_fr[:, b, :], fqn[:, b, :], identity)
        fq_rows = fp.tile([NT * H, 4, P], bf16, tag="fq_rows")
        nc.scalar.copy(out=fq_rows, in_=ps_fr)
        for do in range(2, 2 * H):
            kproj_do(do)

        # ---- k sum-of-squares -> fk (rows r = 4h + mt) ----
        ps_ssk = ps_ss.tile([MT * H, 512], f32, tag="ss")
        n_ssk = H * MT * 2
        i_ssk = 0
        for h in range(H):
            sqt = sq.tile([P, 2, M], bf16, tag="sqk")
            nc.vector.tensor_tensor(sqt, kT[:, 2 * h:2 * h + 2, :],
                                    kT[:, 2 * h:2 * h + 2, :], mult_op)
            for mt in range(MT):
                for dc in range(2):
                    nc.tensor.matmul(
                        ps_ssk, Dk[:, MT * h + mt, :],
                        sqt[:, dc, mt * 512:(mt + 1) * 512],
                        start=(i_ssk == 0), stop=(i_ssk == n_ssk - 1),
                        skip_group_check=True)
                    i_ssk += 1
        ssk_sb = fp.tile([MT * H, 512], f32, tag="ssk_sb")
        nc.scalar.copy(out=ssk_sb, in_=ps_ssk)
        ps_tk = ps_m.tile([P, 4, MT * H], f32, tag="m")
        for b in range(4):
            nc.tensor.transpose(ps_tk[:, b, :],
                                ssk_sb[:, b * P:(b + 1) * P],
                                identity[:MT * H, :MT * H])
        # fk = r_h * SCALE / max(sqrt(ss), eps) in [key-part, b, r] layout
        fkn = fp.tile([P, 4, MT * H], f32, tag="fkn")
        nc.scalar.activation(fkn, ps_tk, mybir.ActivationFunctionType.Sqrt)
        nc.vector.tensor_scalar_max(fkn, fkn, EPS)
        nc.vector.reciprocal(fkn, fkn)
        fkn_v = fkn.rearrange("p b (h mt) -> p b h mt", mt=MT)
        nc.vector.tensor_tensor(
            fkn_v, fkn_v,
            rad_s[:, None, :, None].to_broadcast((P, 4, H, MT)), mult_op)

        # wp load/transpose (needed only by the output projection)
        wp_bf = natp.tile([P, CCH, C], bf16)
        load_cast(wp_t[:].rearrange("(oo p) c -> p oo c", p=P), wp_bf, CCH)
        WpT = wpool.tile([P, CCH, C], bf16)
        transpose_chunks(wp_bf, WpT, CCH)

        # scale qT in place: q_hat = q_raw * fq. fq is broadcast across
        # partitions with a 1-contraction ones matmul (no DRAM bounce).
        for h in range(H):
            for nt in range(NT):
                ps_b = ps_m.tile([P, 512], f32, tag="m",
                                 name=f"psb{h}{nt}")
                nc.tensor.matmul(
                    ps_b, Dbc[:, 2 * h + nt, :],
                    fq_rows.rearrange("r b p -> r (b p)"),
                    start=True, stop=True)
                nc.vector.tensor_tensor(
                    qT[:, 2 * h:2 * h + 2, nt * 512:(nt + 1) * 512],
                    qT[:, 2 * h:2 * h + 2, nt * 512:(nt + 1) * 512],
                    ps_b[:, None, :].to_broadcast((P, 2, 512)), mult_op)

        outcatT = big.tile([P, H, N_CORE], bf16, tag="ocT")
        rdens = [None, None]

        def outproj_block(nt, b):
            # outcatT is already normalized: plain psum accumulation over h
            nn = nt * 4 + b
            ps_o = ps_m.tile([P, C], f32, tag="m", name=f"pso{nt}{b}")
            for h in range(H):
                nc.tensor.matmul(ps_o, outcatT[:, h, nn * P:(nn + 1) * P],
                                 WpT[:, h, :], start=(h == 0),
                                 stop=(h == H - 1))
            acc_o = outp.tile([P, C], f32, tag="acco")
            nc.scalar.copy(out=acc_o, in_=ps_o)
            nc.scalar.dma_start(out=out_t[nn * P:(nn + 1) * P, :],
                                in_=acc_o)

        def scale_outcat(nt):
            # outcatT[:, h, nt half] *= 1/den broadcast across partitions
            rden = rdens[nt]
            ps_rr = ps_m.tile([H, 4, P], f32, tag="m", name=f"psrr{nt}")
            for b in range(4):
                nc.tensor.transpose(ps_rr[:, b, :], rden[:, b, :], identity)
            rr = fp.tile([H, 4, P], bf16, tag="rr", name=f"rr{nt}")
            nc.scalar.copy(out=rr, in_=ps_rr)
            for h in range(H):
                ps_b = ps_m.tile([P, 512], f32, tag="m",
                                 name=f"psbo{nt}{h}")
                nc.tensor.matmul(ps_b, Dbc4[:, h, :],
                                 rr.rearrange("r b p -> r (b p)"),
                                 start=True, stop=True)
                nc.vector.tensor_tensor(
                    outcatT[:, h, nt * 512:(nt + 1) * 512],
                    outcatT[:, h, nt * 512:(nt + 1) * 512],
                    ps_b, mult_op)

        def flash_half(nt, after_head=None):
            den_ps = ps_ss.tile([H, 512], f32, tag="ss", name=f"den{nt}")
            for h in range(H):
                avo = ps_av.tile([P, 512], f32, tag="av", name=f"avo{nt}{h}")
                acc = accp.tile([P, 512], f16, tag="acc",
                                name=f"acc{nt}{h}")
                for j in range(MM):
                    psS = ps_s.tile([P, 512], f32, tag="s")
                    nc.tensor.matmul(psS, kT[:, 2 * h, j * P:(j + 1) * P],
                                     qT[:, 2 * h, nt * 512:(nt + 1) * 512],
                                     start=True, stop=False)
                    nc.tensor.matmul(psS,
                                     kT[:, 2 * h + 1, j * P:(j + 1) * P],
                                     qT[:, 2 * h + 1,
                                        nt * 512:(nt + 1) * 512],
                                     start=False, stop=True)
                    PT = ptp.tile([P, 512], f16, tag="pt")
                    nc.scalar.activation(
                        PT, psS, mybir.ActivationFunctionType.Exp,
                        bias=bias_t[:, 0:1],
                        scale=fkn[:, j % 4, MT * h + j // 4:MT * h + j // 4
                                  + 1])
                    nc.tensor.matmul(avo, v_sb[:, j, h * P:(h + 1) * P], PT,
                                     start=(j == 0), stop=(j == MM - 1))
                    # key-chunk partial of the softmax denominator on DVE
                    # (fp16 sbuf-only ops hit the 2x mode)
                    if j == 0:
                        nc.vector.tensor_copy(out=acc, in_=PT)
                    else:
                        nc.vector.tensor_tensor(acc, acc, PT, add_op)
                nc.tensor.matmul(den_ps, Dden[:, h, :], acc,
                                 start=(h == 0), stop=(h == H - 1),
                                 skip_group_check=True)
                nc.vector.tensor_copy(
                    out=outcatT[:, h, nt * 512:(nt + 1) * 512], in_=avo)
                if after_head is not None:
                    after_head(h)
            # 1/den, transposed to natural [n] per-partition orientation
            den_sb = fp.tile([H, 512], f32, tag="den_sb", name=f"densb{nt}")
            nc.scalar.copy(out=den_sb, in_=den_ps)
            ps_td = ps_m.tile([P, 4, H], f32, tag="m", name=f"pstd{nt}")
            for b in range(4):
                nc.tensor.transpose(ps_td[:, b, :],
                                    den_sb[:, b * P:(b + 1) * P],
                                    identity[:H, :H])
            rden = fp.tile([P, 4, H], f32, tag="rden", name=f"rden{nt}")
            nc.vector.reciprocal(rden, ps_td)
            rdens[nt] = rden

        def after_head_nt1(h):
            if h == 0:
                scale_outcat(0)
            outproj_block(0, h)

        flash_half(0)
        # interleave half 0's normalize+projection into half 1's flash
        flash_half(1, after_head=after_head_nt1)
        scale_outcat(1)
        for b in range(4):
            outproj_block(1, b)

    nc.compile()
    return nc


def _get_nc(mm_bf16: bool):
    if mm_bf16 not in _NC_CACHE:
        _NC_CACHE[mm_bf16] = _build(mm_bf16)
    return _NC_CACHE[mm_bf16]


def kernel(x, context, W_qkv, W_proj, radius, _trace=False, _bf16=True):
    from concourse.bass_utils import run_bass_kernel_spmd

    x = np.ascontiguousarray(np.asarray(x, dtype=np.float32))
    context = np.ascontiguousarray(np.asarray(context, dtype=np.float32))
    W_qkv = np.ascontiguousarray(np.asarray(W_qkv, dtype=np.float32))
    W_proj = np.ascontiguousarray(np.asarray(W_proj, dtype=np.float32))
    radius = np.ascontiguousarray(np.asarray(radius, dtype=np.float32))

    nc = _get_nc(True)
    in_maps = []
    for i in range(8):
        b, half = i // 2, i % 2
        in_maps.append({
            "x_sh": x[b, half * N_CORE:(half + 1) * N_CORE, :],
            "ctx": context[b],
            "w_qkv": W_qkv,
            "w_proj": W_proj,
            "radius": radius,
        })
    res = run_bass_kernel_spmd(nc, in_maps, list(range(8)), trace=_trace)
    out = np.empty((B, N, C), dtype=np.float32)
    for i in range(8):
        b, half = i // 2, i % 2
        out[b, half * N_CORE:(half + 1) * N_CORE, :] = res.results[i]["out_sh"]
    if _trace:
        return out, res
    return out
